# revision 5
# baseline (speedup 1.0000x reference)
"""Causal multi-head attention block on 8 Trainium2 NeuronCores.

Problem: B=4, S=2048, D=1024, H=16 heads (d_k=64), causal softmax attention
with Q/K/V/O projections (torch Linear convention: y = x @ W.T + b).

Sharding: 2-way tensor parallel over heads x 4-way data parallel over batch.
Core c handles batch b = c // 2 and head group g = c % 2 (8 heads, 512
features). Each core computes its partial out-projection; the host sums the
two partials per batch and adds the bias constant (bo + bv @ Wo.T — the V
bias contributes a constant row because softmax rows sum to 1).

Per-core kernel (all matmuls on the PE in fp32r, full speed at N=512):
  Stage A: QT, KT [512f, 2048s] and V [2048s, 512f] via projection matmuls.
           V is stored as 16 tiles [128, 8 heads * 65] with a ones column
           per head (the ones column makes the PV matmul emit softmax
           denominators as row 64 of its PSUM output).
  Attention, per (head, q-tile of 512): S^T blocks [k=128, q=512] =
           KT_h.T @ QT_h on the PE; exp on ACT (no row-max — scores are
           O(1) by construction); causal masking of diagonal blocks via
           gpsimd affine_select (fill 0 after exp); PV accumulate
           attnT_un[65, 512] over k-chunks; normalize with DVE reciprocal +
           gpsimd partition_broadcast + DVE multiply.
  Out-proj, per s-tile of 128: y[s, o] accumulated over the 4 f-chunks,
           copied to SBUF and DMA'd out.
"""

import math

import ml_dtypes
import numpy as np

import concourse.bass as bass
import concourse.mybir as mybir
import concourse.tile as tile
from concourse import bacc
from concourse.bass_utils import run_bass_kernel_spmd

F32 = mybir.dt.float32
F32R = mybir.dt.float32r
BF = mybir.dt.bfloat16
AF = mybir.ActivationFunctionType
ALU = mybir.AluOpType

N_CORES = 8
S = 2048
D = 1024
H = 16
DK = 64
HPC = 8          # heads per core
FC = HPC * DK    # features per core = 512
ND = D // 128    # d_model chunks of 128 = 8
NF = FC // 128   # feature tiles of 128 = 4
NQ = S // 512    # q tiles of 512 = 4
NS1 = S // 128   # s tiles of 128 = 16


def emit_kernel_body(tc, xT, wqT, wkT, wvT, woT, bq, bk, out):
    """Emit the per-core attention kernel IR into TileContext tc.

    All arguments are DRAM APs:
      xT  [1024, 2048]  x[b].T
      wqT/wkT/wvT [1024, 512]   W[slice].T (d_in rows, features cols)
      woT [512, 1024]           Wo[:, slice].T (features rows, d_out cols)
      bq/bk [128, 4]            biases, column j = features j*128..j*128+127
                                (bq NOT pre-scaled; the 1/sqrt(dk) scale is
                                applied here)
      out [2048, 1024]          partial output for this batch
    """
    nc = tc.nc
    with (
        tc.tile_pool(name="w8k", bufs=6) as w8k,        # 6 x 8KB
        tc.tile_pool(name="x4k", bufs=8) as x4k,        # 8 x 4KB
        tc.tile_pool(name="qt8k", bufs=8) as qt8k,      # 8 x 8KB
        tc.tile_pool(name="v520", bufs=16) as v520,     # 16 x ~2KB
        tc.tile_pool(name="small", bufs=4) as small,
        tc.tile_pool(name="ps1k", bufs=2, space="PSUM") as ps1k,
        tc.tile_pool(name="ps512", bufs=4, space="PSUM") as ps512,
    ):
        # ---- load weights and biases ----
        def load_w2(dram, tag):
            tiles = []
            for half in range(2):
                t = w8k.tile([128, 2048], BF, tag=tag)
                src = dram[half * 512 : (half + 1) * 512, :].rearrange(
                    "(c p) f -> p c f", p=128
                )
                nc.sync.dma_start(t[:].rearrange("p (c f) -> p c f", f=512), src)
                tiles.append(t)
            return tiles

        wq = load_w2(wqT, "w8k")
        wk = load_w2(wkT, "w8k")
        wv = load_w2(wvT, "w8k")

        bq_sb = small.tile([128, 4], F32, tag="bias")
        nc.sync.dma_start(bq_sb[:], bq[:])
        bk_sb = small.tile([128, 4], F32, tag="bias")
        nc.sync.dma_start(bk_sb[:], bk[:])

        qt = [qt8k.tile([128, 2048], BF, tag="qt8k", name=f"qt{j}") for j in range(NF)]
        kt = [qt8k.tile([128, 2048], BF, tag="qt8k", name=f"kt{j}") for j in range(NF)]
        vaug = []

        scale = 1.0 / math.sqrt(DK)

        # ---- stage A: projections ----
        for i in range(NQ):  # s-tile of 512
            # x tiles: 4 x [128, 1024] (d-chunks 2t, 2t+1)
            xa = []
            for t in range(4):
                xt_t = x4k.tile([128, 1024], BF, tag="x4k")
                src = xT[2 * t * 128 : (2 * t + 2) * 128, i * 512 : (i + 1) * 512]
                nc.sync.dma_start(
                    xt_t[:].rearrange("p (c s) -> p c s", s=512),
                    src.rearrange("(c p) s -> p c s", p=128),
                )
                xa.append(xt_t)

            def xslice(dc, lo=0, n=512):
                return xa[dc // 2][:, (dc % 2) * 512 + lo : (dc % 2) * 512 + lo + n]

            # Q and K projections: psum[f128, s512] accumulated over d
            for wtiles, dst, b_sb, sc in (
                (wq, qt, bq_sb, scale),
                (wk, kt, bk_sb, 1.0),
            ):
                for j in range(NF):
                    ps = ps512.tile([128, 512], F32, tag="ps512")
                    for dc in range(ND):
                        lhsT = wtiles[dc // 4][
                            :, (dc % 4) * 512 + j * 128 : (dc % 4) * 512 + (j + 1) * 128
                        ]
                        nc.tensor.matmul(
                            ps[:],
                            lhsT,
                            xslice(dc),
                            start=(dc == 0),
                            stop=(dc == ND - 1),
                        )
                    # (psum + bias) * sc -> SBUF
                    nc.vector.tensor_scalar(
                        dst[j][:, i * 512 : (i + 1) * 512],
                        ps[:],
                        b_sb[:, j : j + 1],
                        sc,
                        op0=ALU.add,
                        op1=ALU.mult,
                    )

            # V projection: psum[s128, f512]; stored strided with ones cols
            for t in range(4):
                ps = ps512.tile([128, 512], F32, tag="ps512")
                for dc in range(ND):
                    rhs = wv[dc // 4][:, (dc % 4) * 512 : (dc % 4 + 1) * 512]
                    nc.tensor.matmul(
                        ps[:],
                        xslice(dc, t * 128, 128),
                        rhs,
                        start=(dc == 0),
                        stop=(dc == ND - 1),
                    )
                va = v520.tile([128, HPC * 65], BF, tag="v520")
                nc.gpsimd.memset(va[:], 1.0)
                nc.vector.tensor_copy(
                    va[:].rearrange("p (h c) -> p h c", c=65)[:, :, 0:DK],
                    ps[:].rearrange("p (h c) -> p h c", c=DK),
                )
                vaug.append(va)

        # ---- attention ----
        attnT = [w8k.tile([128, 2048], BF, tag="w8k", name=f"attnT{j}") for j in range(NF)]

        for h in range(HPC):
            tj, prow = h // 2, (h % 2) * 64
            for i in range(NQ):
                kmax = 4 * (i + 1)
                pv = ps512.tile([128, 512], F32, tag="ps512")
                for kp in range((kmax + 1) // 2):
                    sps = ps1k.tile([128, 1024], F32, tag="ps1k")
                    ncol = 1024 if 2 * kp + 1 < kmax else 512
                    pt = x4k.tile([128, 1024], BF, tag="x4k")
                    for half in range(ncol // 512):
                        kc = 2 * kp + half
                        nc.tensor.matmul(
                            sps[:, half * 512 : half * 512 + 512],
                            kt[tj][prow : prow + 64, kc * 128 : (kc + 1) * 128],
                            qt[tj][prow : prow + 64, i * 512 : (i + 1) * 512],
                            start=True,
                            stop=True,
                        )
                    nc.scalar.activation(pt[:, 0:ncol], sps[:, 0:ncol], AF.Exp)
                    for half in range(ncol // 512):
                        kc = 2 * kp + half
                        if kc >= 4 * i:  # diagonal block: mask strict lower part
                            m = kc * 128 - i * 512
                            nc.gpsimd.affine_select(
                                out=pt[:, half * 512 : half * 512 + 512],
                                in_=pt[:, half * 512 : half * 512 + 512],
                                compare_op=ALU.is_ge,
                                fill=0.0,
                                base=-m,
                                pattern=[[1, 512]],
                                channel_multiplier=-1,
                            )
                        nc.tensor.matmul(
                            pv[0:65, :],
                            vaug[kc][:, h * 65 : h * 65 + 65],
                            pt[:, half * 512 : half * 512 + 512],
                            start=(kc == 0),
                            stop=(kc == kmax - 1),
                            skip_group_check=True,
                        )
                # normalize: attnT rows = attnT_un / den
                rec = small.tile([1, 512], F32, tag="rec")
                nc.vector.reciprocal(rec[:], pv[64:65, :])
                bc = small.tile([64, 512], F32, tag="bc")
                nc.gpsimd.partition_broadcast(bc[:], rec[:], channels=64)
                nc.vector.tensor_tensor(
                    attnT[tj][prow : prow + 64, i * 512 : (i + 1) * 512],
                    pv[0:64, :],
                    bc[:],
                    op=ALU.mult,
                )

        # ---- out-projection ----
        wo = []
        for half in range(2):
            t = w8k.tile([128, 2048], BF, tag="w8k")
            src = woT[half * 256 : (half + 1) * 256, :]
            # [256, 1024] -> [128, 2, 1024]
            nc.sync.dma_start(
                t[:].rearrange("p (c o) -> p c o", o=1024),
                src.rearrange("(c p) o -> p c o", p=128),
            )
            wo.append(t)

        for t in range(NS1):
            ysb = x4k.tile([128, 1024], F32, tag="x4k")
            for oc in range(2):
                ps = ps512.tile([128, 512], F32, tag="ps512")
                for fc in range(NF):
                    nc.tensor.matmul(
                        ps[:],
                        attnT[fc][:, t * 128 : (t + 1) * 128],
                        wo[fc // 2][
                            :, (fc % 2) * 1024 + oc * 512 : (fc % 2) * 1024 + oc * 512 + 512
                        ],
                        start=(fc == 0),
                        stop=(fc == NF - 1),
                    )
                nc.vector.tensor_copy(ysb[:, oc * 512 : oc * 512 + 512], ps[:])
            nc.sync.dma_start(out[t * 128 : (t + 1) * 128, :], ysb[:])


def build_nc(reps=1):
    nc = bacc.Bacc(
        "TRN2", target_bir_lowering=False, debug=False, num_devices=N_CORES
    )
    xT = nc.dram_tensor("xT", [D, S], BF, kind="ExternalInput").ap()
    wqT = nc.dram_tensor("wqT", [D, FC], BF, kind="ExternalInput").ap()
    wkT = nc.dram_tensor("wkT", [D, FC], BF, kind="ExternalInput").ap()
    wvT = nc.dram_tensor("wvT", [D, FC], BF, kind="ExternalInput").ap()
    woT = nc.dram_tensor("woT", [FC, D], BF, kind="ExternalInput").ap()
    bq = nc.dram_tensor("bq", [128, NF], F32, kind="ExternalInput").ap()
    bk = nc.dram_tensor("bk", [128, NF], F32, kind="ExternalInput").ap()
    out = nc.dram_tensor("out", [S, D], F32, kind="ExternalOutput").ap()

    with tile.TileContext(nc) as tc:
        if reps == 1:
            emit_kernel_body(tc, xT, wqT, wkT, wvT, woT, bq, bk, out)
        else:
            with tc.For_i(0, reps, 1):
                emit_kernel_body(tc, xT, wqT, wkT, wvT, woT, bq, bk, out)
    nc.finalize()
    return nc


def make_in_maps(x, Wq, bq, Wk, bk, Wv, bv, Wo, bo):
    in_maps = []
    for c in range(N_CORES):
        b, g = c // 2, c % 2
        sl = slice(g * FC, (g + 1) * FC)
        in_maps.append(
            {
                "xT": np.ascontiguousarray(x[b].T).astype(ml_dtypes.bfloat16),
                "wqT": np.ascontiguousarray(Wq[sl, :].T).astype(ml_dtypes.bfloat16),
                "wkT": np.ascontiguousarray(Wk[sl, :].T).astype(ml_dtypes.bfloat16),
                "wvT": np.ascontiguousarray(Wv[sl, :].T).astype(ml_dtypes.bfloat16),
                "woT": np.ascontiguousarray(Wo[:, sl].T).astype(ml_dtypes.bfloat16),
                "bq": np.ascontiguousarray(bq[sl].reshape(NF, 128).T),
                "bk": np.ascontiguousarray(bk[sl].reshape(NF, 128).T),
            }
        )
    return in_maps


def assemble_output(per_core_outs, bv, Wo, bo):
    const = (bv @ Wo.T + bo).astype(np.float32)
    y = np.empty((4, S, D), np.float32)
    for b in range(4):
        y[b] = per_core_outs[2 * b] + per_core_outs[2 * b + 1] + const
    return y


def kernel(**inputs):
    inputs = {k: np.asarray(v, dtype=np.float32) for k, v in inputs.items()}
    nc = build_nc(reps=1)
    in_maps = make_in_maps(
        inputs["x"], inputs["Wq"], inputs["bq"], inputs["Wk"], inputs["bk"],
        inputs["Wv"], inputs["bv"], inputs["Wo"], inputs["bo"],
    )
    res = run_bass_kernel_spmd(nc, in_maps, core_ids=list(range(N_CORES)))
    outs = [res.results[c]["out"] for c in range(N_CORES)]
    return assemble_output(outs, inputs["bv"], inputs["Wo"], inputs["bo"])
